# revision 32
# baseline (speedup 1.0000x reference)
"""Trainium2 Bass kernel for nn_DNALayer (block-local attn + global bucket attn + FFN).

Self-contained: accepts FULL inputs, shards tokens 8-way across NeuronCores
(block-diagonal local attention makes 128-token blocks independent), runs one
SPMD Bass program per core, gathers the full output.

Layout: activations feature-major [128 part, 6 fchunks, 512 tok] in SBUF.
All big matmuls in float32r (1 cyc/row at N>=256, ~1e-4 rel precision).
"""
import os
from contextlib import ExitStack

import numpy as np

import concourse.bacc as bacc
import concourse.bass as bass
import concourse.tile as tile
import concourse.masks as masks
from concourse import mybir
from concourse.bass_utils import run_bass_kernel_spmd

F32 = mybir.dt.float32
F32R = mybir.dt.float32r
BF16 = mybir.dt.bfloat16
AF = mybir.ActivationFunctionType
OP = mybir.AluOpType

B, S, D = 4, 8192, 768
H, WBLK, NB = 12, 128, 512
DH = 64
EPS = 1e-5
NCORES = 8
T = (B * S) // NCORES          # 4096 tokens per core
TT = 512                       # token tile
NTT = T // TT                  # 8 tiles
FC = D // 128                  # 6 feature chunks
SCALE = 0.125                  # 1/sqrt(DH)

WEIGHT_NAMES = [
    "bucket_matrix",
    "lq_w", "lq_b", "lk_w", "lk_b", "lv_w", "lv_b", "lo_w", "lo_b",
    "gq_w", "gq_b", "gkv_w", "gkv_b", "go_w", "go_b",
    "f1_w", "f1_b", "f2_w", "f2_b",
    "n1_g", "n1_b", "n2_g", "n2_b", "n3_g", "n3_b",
]


TAP = None
DBG_D = None


def _tap(nc, name, ap, tt):
    if TAP == name and tt == 0:
        nc.sync.dma_start(DBG_D.ap(), ap.bitcast(F32))


def _hrows(hh):
    """Head hh occupies feature rows (pbase..pbase+64) of chunk fc."""
    return (hh % 2) * 64, hh // 2


def _load_w_chunks(nc, pool, w_d, ncols, name):
    """Load a [768, ncols] f32r weight into 6 chunk tiles [128, ncols]."""
    tiles = []
    for kc in range(FC):
        t = pool.tile([128, ncols], F32R, name=f"{name}{kc}", tag=f"{name}{kc}")
        nc.sync.dma_start(t[:], w_d.ap()[kc * 128:(kc + 1) * 128, :])
        tiles.append(t)
    return tiles


def _load_bias(nc, pool, b_d, nchunks, name):
    """Load a [128*nchunks] bias into [128, nchunks] (col c = rows c*128..)."""
    t = pool.tile([128, nchunks], F32, name=name, tag=name)
    nc.sync.dma_start(t[:], b_d.ap().rearrange("(c p) -> p c", p=128))
    return t


def build(t_total=T, tap=None):
    global T, NTT, TAP
    TAP = tap
    T, NTT = t_total, t_total // TT
    nc = bacc.Bacc("TRN2", target_bir_lowering=False, debug=False,
                   num_devices=NCORES)

    x_d = nc.dram_tensor("x", [T, D], F32, kind="ExternalInput")
    wd = {}
    for nm, shape, dt in [
        ("bucket_matrix", [NB, D], F32),
        ("lq_w", [D, D], F32R), ("lk_w", [D, D], F32R),
        ("lv_w", [D, D], F32R), ("lo_w", [D, D], F32R),
        ("gq_w", [D, D], F32R), ("gkv_w", [D, 2 * D], F32R),
        ("go_w", [D, D], F32R),
        ("f1_w", [D, 4 * D], F32R), ("f2_w", [4 * D, D], F32R),
        ("lq_b", [D], F32), ("lo_bp", [D], F32),
        ("gq_b", [D], F32), ("go_bp", [D], F32),
        ("f1_b", [4 * D], F32), ("f2_b", [D], F32),
        ("n1_g", [D], F32), ("n1_b", [D], F32),
        ("n2_g", [D], F32), ("n2_b", [D], F32),
        ("n3_g", [D], F32), ("n3_b", [D], F32),
    ]:
        wd[nm] = nc.dram_tensor(nm, shape, dt, kind="ExternalInput")
    out_d = nc.dram_tensor("out", [T, D], F32, kind="ExternalOutput")
    global DBG_D
    DBG_D = nc.dram_tensor("dbg", [128, 3072], F32,
                           kind="ExternalOutput") if TAP else None

    with tile.TileContext(nc) as tc:
        with ExitStack() as octx:
            dpool = octx.enter_context(
                tc.tile_pool(name="spill", bufs=1, space="DRAM"))
            y1_d = dpool.tile([FC, 128, T], F32R, name="y1s", tag="y1s")
            y2_d = dpool.tile([FC, 128, T], F32R, name="y2s", tag="y2s")
            cpool = octx.enter_context(tc.tile_pool(name="consts", bufs=1))
            ident = cpool.tile([128, 128], F32)
            masks.make_identity(nc, ident[:])
            ident_r = cpool.tile([128, 128], F32R)
            nc.vector.tensor_copy(ident_r[:], ident[:])
            ident_b = cpool.tile([128, 128], BF16)
            nc.vector.tensor_copy(ident_b[:], ident[:])
            cmask = cpool.tile([128, 128], F32)
            nc.gpsimd.memset(cmask[:], 0.0)
            # additive mask, q-major: keep 0 where q - k >= 0, else -1e5
            nc.gpsimd.affine_select(
                out=cmask[:], in_=cmask[:], compare_op=OP.is_ge,
                fill=-1.0e5, base=0, pattern=[[-1, 128]], channel_multiplier=1)
            ones_f = cpool.tile([128, 1], F32)
            nc.gpsimd.memset(ones_f[:], 1.0)
            ones_r = cpool.tile([128, 1], F32R)
            nc.vector.tensor_copy(ones_r[:], ones_f[:])
            ones64b = cpool.tile([1, 64], BF16)
            nc.gpsimd.memset(ones64b[:], 1.0)
            eps_row = cpool.tile([1, 1], F32)
            nc.gpsimd.memset(eps_row[:], EPS)
            eps_col = cpool.tile([128, 1], F32)
            nc.gpsimd.memset(eps_col[:], EPS)

            n1g = _load_bias(nc, cpool, wd["n1_g"], FC, "n1g")
            n1b = _load_bias(nc, cpool, wd["n1_b"], FC, "n1b")
            n2g = _load_bias(nc, cpool, wd["n2_g"], FC, "n2g")
            n2b = _load_bias(nc, cpool, wd["n2_b"], FC, "n2b")
            # n3 gain/bias broadcast along partitions (token-major apply)
            n3rowg = cpool.tile([1, D], F32, name="n3rowg", tag="n3row")
            nc.sync.dma_start(n3rowg[:], wd["n3_g"].ap()[None, :])
            g3bc = cpool.tile([128, D], F32)
            nc.gpsimd.partition_broadcast(g3bc[:], n3rowg[:])
            n3rowb = cpool.tile([1, D], F32, name="n3rowb", tag="n3rowb")
            nc.sync.dma_start(n3rowb[:], wd["n3_b"].ap()[None, :])
            b3bc = cpool.tile([128, D], F32)
            nc.gpsimd.partition_broadcast(b3bc[:], n3rowb[:])

            _phase1(nc, tc, octx, x_d, wd, y1_d,
                    ident, ident_b, cmask, ones_r, eps_row, n1g, n1b)
            _phase2(nc, tc, octx, wd, y1_d, y2_d,
                    ident, ones_r, ones64b, eps_row, n2g, n2b)
            _phase3(nc, tc, octx, wd, y2_d, out_d,
                    ident, ident_r, ones_r, eps_row, g3bc, b3bc)

    nc.compile()
    return nc


def _proj_fm(nc, ppsum, wtiles, src, dst, bias_ap=None, scale=None):
    """dst[:, fo, :] = act(W^T @ src + bias); feature-major in/out."""
    for fo in range(FC):
        ps = ppsum.tile([128, TT], F32, name=f"pp{fo}", tag="proj")
        for kc in range(FC):
            nc.tensor.matmul(ps[:], wtiles[kc][:, fo * 128:(fo + 1) * 128],
                             src[:, kc, :], start=(kc == 0), stop=(kc == FC - 1))
        if bias_ap is not None:
            if fo % 2 == 0:
                nc.scalar.activation(dst[:, fo, :], ps[:], AF.Identity,
                                     bias=bias_ap[:, fo:fo + 1])
            else:
                nc.vector.tensor_scalar(
                    out=dst[:, fo, :], in0=ps[:],
                    scalar1=bias_ap[:, fo:fo + 1], scalar2=None, op0=OP.add)
        else:
            nc.any.tensor_copy(dst[:, fo, :], ps[:])


def _ln_fm(nc, tc, pools, rT, yT, g_sb, b_sb, ones_r, eps_row):
    """Feature-major LayerNorm over features via PE ones-matmuls."""
    ppsum, rowp, sqp, bcp, tmpp = pools
    Sps = ppsum.tile([1, TT], F32, name="lnS", tag="proj")
    Qps = ppsum.tile([1, TT], F32, name="lnQ", tag="proj")
    for kc in range(FC):
        sq = sqp.tile([128, TT], F32R, name="lnsq", tag="lnsq")
        nc.vector.tensor_tensor(sq[:], rT[:, kc, :], rT[:, kc, :], op=OP.mult)
        nc.tensor.matmul(Sps[:], ones_r[:], rT[:, kc, :],
                         start=(kc == 0), stop=(kc == FC - 1))
        nc.tensor.matmul(Qps[:], ones_r[:], sq[:],
                         start=(kc == 0), stop=(kc == FC - 1))
    mu = rowp.tile([1, TT], F32, name="lnmu", tag="lnmu")
    nc.scalar.mul(mu[:], Sps[:], 1.0 / D)
    msq = rowp.tile([1, TT], F32, name="lnmsq", tag="lnmsq")
    nc.scalar.mul(msq[:], Qps[:], 1.0 / D)
    mu2 = rowp.tile([1, TT], F32, name="lnmu2", tag="lnmu2")
    nc.vector.tensor_tensor(mu2[:], mu[:], mu[:], op=OP.mult)
    varr = rowp.tile([1, TT], F32, name="lnvar", tag="lnvar")
    nc.vector.tensor_tensor(varr[:], msq[:], mu2[:], op=OP.subtract)
    lnv = rowp.tile([1, TT], F32, name="lnln", tag="lnln")
    istd = rowp.tile([1, TT], F32, name="lnistd", tag="lnistd")
    nc.scalar.activation(lnv[:], varr[:], AF.Ln, bias=eps_row[:])
    nc.scalar.activation(istd[:], lnv[:], AF.Exp, scale=-0.5)
    mubc = bcp.tile([128, TT], F32, name="lnmubc", tag="lnmubc")
    nc.gpsimd.partition_broadcast(mubc[:], mu[:])
    ubc = bcp.tile([128, TT], F32, name="lnubc", tag="lnubc")
    nc.gpsimd.partition_broadcast(ubc[:], istd[:])
    for kc in range(FC):
        d = tmpp.tile([128, TT], F32, name="lnd", tag="lnd")
        nc.vector.tensor_tensor(d[:], rT[:, kc, :], mubc[:], op=OP.subtract)
        e = tmpp.tile([128, TT], F32, name="lne", tag="lne")
        nc.vector.tensor_tensor(e[:], d[:], ubc[:], op=OP.mult)
        nc.scalar.activation(yT[:, kc, :], e[:], AF.Identity,
                             bias=b_sb[:, kc:kc + 1], scale=g_sb[:, kc:kc + 1])


def _phase1(nc, tc, octx, x_d, wd, y1_d, ident, ident_b, cmask, ones_r,
            eps_row, n1g, n1b):
    with ExitStack() as ctx:
        wpool = ctx.enter_context(tc.tile_pool(name="p1w", bufs=1))
        apool = ctx.enter_context(tc.tile_pool(name="p1a", bufs=1))
        inpool = ctx.enter_context(tc.tile_pool(name="p1in", bufs=1))
        empool = ctx.enter_context(tc.tile_pool(name="p1em", bufs=3))
        rowp = ctx.enter_context(tc.tile_pool(name="p1row", bufs=1))
        bcp = ctx.enter_context(tc.tile_pool(name="p1bc", bufs=1))
        sqp = ctx.enter_context(tc.tile_pool(name="p1sq", bufs=2))
        tmpp = ctx.enter_context(tc.tile_pool(name="p1tmp", bufs=1))
        ppsum = ctx.enter_context(
            tc.tile_pool(name="p1pp", bufs=2, space="PSUM"))
        spsum = ctx.enter_context(
            tc.tile_pool(name="p1sp", bufs=4, space="PSUM"))
        avpsum = ctx.enter_context(
            tc.tile_pool(name="p1av", bufs=1, space="PSUM"))

        lqw = _load_w_chunks(nc, wpool, wd["lq_w"], D, "lqw")
        lkw = _load_w_chunks(nc, wpool, wd["lk_w"], D, "lkw")
        lvw = _load_w_chunks(nc, wpool, wd["lv_w"], D, "lvw")
        low = _load_w_chunks(nc, wpool, wd["lo_w"], D, "low")
        lqb = _load_bias(nc, wpool, wd["lq_b"], FC, "lqb")
        lobp = _load_bias(nc, wpool, wd["lo_bp"], FC, "lobp")

        for tt in range(NTT):
            x_tm = inpool.tile([128, 4, D], F32, name="xtm", tag="xtm")
            nc.sync.dma_start(
                x_tm[:],
                x_d.ap()[tt * TT:(tt + 1) * TT, :].rearrange(
                    "(b p) d -> p b d", p=128))
            xT = apool.tile([128, FC, TT], F32R, name="xT", tag="xT")
            for b in range(4):
                for fc in range(FC):
                    tps = spsum.tile([128, 128], F32, name="xtr", tag="sblk")
                    nc.tensor.transpose(
                        tps[:], x_tm[:, b, fc * 128:(fc + 1) * 128], ident[:])
                    nc.any.tensor_copy(xT[:, fc, b * 128:(b + 1) * 128], tps[:])

            _tap(nc, "xT", xT[:].rearrange("p a b -> p (a b)"), tt)
            qT = apool.tile([128, FC, TT], BF16, name="qT", tag="qT")
            _proj_fm(nc, ppsum, lqw, xT, qT, bias_ap=lqb)
            kT = apool.tile([128, FC, TT], BF16, name="kT", tag="kT")
            _proj_fm(nc, ppsum, lkw, xT, kT)
            v_tm = apool.tile([128, 4, D], BF16, name="vtm", tag="vtm")
            for b in range(4):
                for half in range(2):
                    ps = ppsum.tile([128, 384], F32, name="vp", tag="proj")
                    for kc in range(FC):
                        nc.tensor.matmul(
                            ps[:], xT[:, kc, b * 128:(b + 1) * 128],
                            lvw[kc][:, half * 384:(half + 1) * 384],
                            start=(kc == 0), stop=(kc == FC - 1))
                    nc.any.tensor_copy(
                        v_tm[:, b, half * 384:(half + 1) * 384], ps[:])

            aoT = apool.tile([128, FC, TT], F32R, name="aoT", tag="aoT")
            for b in range(4):
                avps = avpsum.tile([128, FC, 128], F32, name="avps", tag="av")
                nc.vector.memset(avps[:], 0.0)
                zb = rowp.tile([128, H], F32, name="zb", tag="zb")
                ems = []
                for hh in range(H):
                    pb, fc = _hrows(hh)
                    sps = spsum.tile([128, 128], F32, name="sps", tag="sblk")
                    nc.tensor.matmul(
                        sps[:],
                        qT[pb:pb + 64, fc, b * 128:(b + 1) * 128],
                        kT[pb:pb + 64, fc, b * 128:(b + 1) * 128],
                        start=True, stop=True)
                    sm = empool.tile([128, 128], F32, name="sm", tag="sm",
                                     bufs=2)
                    nc.vector.tensor_tensor(sm[:], sps[:], cmask[:], op=OP.add)
                    em = empool.tile([128, 128], F32, name="em", tag="em",
                                     bufs=13)
                    nc.scalar.activation(em[:], sm[:], AF.Exp, scale=SCALE,
                                         accum_out=zb[:, hh:hh + 1])
                    ems.append(em)
                rzb = rowp.tile([128, H], F32, name="rzb", tag="rzb")
                nc.vector.reciprocal(rzb[:], zb[:])
                for hh in range(H):
                    pb, fc = _hrows(hh)
                    an = empool.tile([128, 128], BF16, name="an", tag="an",
                                     bufs=2)
                    nc.vector.tensor_scalar(
                        out=an[:], in0=ems[hh][:], scalar1=rzb[:, hh:hh + 1],
                        scalar2=None, op0=OP.mult)
                    tps = spsum.tile([128, 128], BF16, name="atp", tag="sblk")
                    nc.tensor.transpose(tps[:], an[:], ident_b[:])
                    anT = empool.tile([128, 128], BF16, name="anT", tag="anT",
                                      bufs=2)
                    nc.any.tensor_copy(anT[:], tps[:])
                    # avps spans 2 psum banks (fc 0-3, fc 4-5); start=True
                    # zeroes a whole bank, so only the first matmul touching
                    # each bank may set it.
                    nc.tensor.matmul(
                        avps[pb:pb + 64, fc, :],
                        v_tm[:, b, hh * 64:(hh + 1) * 64], anT[:],
                        start=False, stop=True, skip_group_check=True)
                nc.any.tensor_copy(aoT[:, :, b * 128:(b + 1) * 128], avps[:])

            _tap(nc, "kT", kT[:].rearrange("p a b -> p (a b)"), tt)
            _tap(nc, "vtm", v_tm[:].rearrange("p a b -> p (a b)").bitcast(F32), tt)
            _tap(nc, "aoT", aoT[:].rearrange("p a b -> p (a b)"), tt)
            r1T = apool.tile([128, FC, TT], F32R, name="r1T", tag="vtm")
            for fo in range(FC):
                ps = ppsum.tile([128, TT], F32, name="lop", tag="proj")
                for kc in range(FC):
                    nc.tensor.matmul(ps[:],
                                     low[kc][:, fo * 128:(fo + 1) * 128],
                                     aoT[:, kc, :],
                                     start=(kc == 0), stop=(kc == FC - 1))
                t1 = tmpp.tile([128, TT], F32, name="lot1", tag="lot1",
                                bufs=2)
                nc.scalar.activation(t1[:], ps[:], AF.Identity,
                                     bias=lobp[:, fo:fo + 1])
                nc.vector.tensor_tensor(r1T[:, fo, :], t1[:], xT[:, fo, :],
                                        op=OP.add)

            _tap(nc, "r1T", r1T[:].rearrange("p a b -> p (a b)"), tt)
            y1T = apool.tile([128, FC, TT], F32R, name="y1T", tag="xT")
            _ln_fm(nc, tc, (ppsum, rowp, sqp, bcp, tmpp), r1T, y1T,
                   n1g, n1b, ones_r, eps_row)
            _tap(nc, "y1T", y1T[:].rearrange("p a b -> p (a b)"), tt)
            for fc in range(FC):
                nc.sync.dma_start(
                    y1_d[fc, :, tt * TT:(tt + 1) * TT], y1T[:, fc, :])


def _phase2(nc, tc, octx, wd, y1_d, y2_d, ident, ones_r, ones64b, eps_row,
            n2g, n2b):
    with ExitStack() as ctx:
        wpool = ctx.enter_context(tc.tile_pool(name="p2w", bufs=1))
        ppsum = ctx.enter_context(
            tc.tile_pool(name="p2pp", bufs=3, space="PSUM"))
        spsum = ctx.enter_context(
            tc.tile_pool(name="p2sp", bufs=2, space="PSUM"))
        avpsum = ctx.enter_context(
            tc.tile_pool(name="p2av", bufs=2, space="PSUM"))

        gqw = _load_w_chunks(nc, wpool, wd["gq_w"], D, "gqw")
        gow = _load_w_chunks(nc, wpool, wd["go_w"], D, "gow")
        gqb = _load_bias(nc, wpool, wd["gq_b"], FC, "gqb")
        gobp = _load_bias(nc, wpool, wd["go_bp"], FC, "gobp")

        # bucket prep: bT (feature-major transposed buckets), kTg, vg1
        kTg = wpool.tile([128, FC, NB], BF16, name="kTg", tag="kTg")
        vg1 = wpool.tile([128, 4, H, 65], BF16, name="vg1", tag="vg1")
        ones48 = wpool.tile([128, 48], F32, name="ones48", tag="ones48")
        nc.gpsimd.memset(ones48[:], 1.0)
        nc.vector.tensor_copy(
            vg1[:, :, :, 64:65],
            ones48[:].rearrange("p (a b c) -> p a b c", a=4, b=H))
        with ExitStack() as pctx:
            gkvpool = pctx.enter_context(tc.tile_pool(name="p2gkv", bufs=2))
            bpool = pctx.enter_context(tc.tile_pool(name="p2b", bufs=1))
            bk_tm = bpool.tile([128, 4, D], F32, name="bktm", tag="bktm")
            nc.sync.dma_start(
                bk_tm[:],
                wd["bucket_matrix"].ap().rearrange("(b p) d -> p b d", p=128))
            bT = bpool.tile([128, FC, NB], F32R, name="bT", tag="bT")
            for b in range(4):
                for fc in range(FC):
                    tps = spsum.tile([128, 128], F32, name="btr", tag="sblk")
                    nc.tensor.transpose(
                        tps[:], bk_tm[:, b, fc * 128:(fc + 1) * 128], ident[:])
                    nc.any.tensor_copy(bT[:, fc, b * 128:(b + 1) * 128], tps[:])
            gkvw = _load_w_chunks(nc, gkvpool, wd["gkv_w"], 2 * D, "gkvw")
            for fo in range(FC):
                ps = ppsum.tile([128, NB], F32, name="kgp", tag="proj")
                for kc in range(FC):
                    nc.tensor.matmul(ps[:],
                                     gkvw[kc][:, fo * 128:(fo + 1) * 128],
                                     bT[:, kc, :],
                                     start=(kc == 0), stop=(kc == FC - 1))
                nc.scalar.copy(kTg[:, fo, :], ps[:])
            for bc in range(4):
                for half in range(2):
                    ps = ppsum.tile([128, 384], F32, name="vgp", tag="proj")
                    for kc in range(FC):
                        nc.tensor.matmul(
                            ps[:], bT[:, kc, bc * 128:(bc + 1) * 128],
                            gkvw[kc][:, D + half * 384:D + (half + 1) * 384],
                            start=(kc == 0), stop=(kc == FC - 1))
                    nc.any.tensor_copy(
                        vg1[:, bc, half * 6:(half + 1) * 6, 0:64], ps[:])

        apool = ctx.enter_context(tc.tile_pool(name="p2a", bufs=1))
        inpool = ctx.enter_context(tc.tile_pool(name="p2in", bufs=2))
        empool = ctx.enter_context(tc.tile_pool(name="p2em", bufs=2))
        rowp = ctx.enter_context(tc.tile_pool(name="p2row", bufs=1))
        bcp = ctx.enter_context(tc.tile_pool(name="p2bc", bufs=1))
        sqp = ctx.enter_context(tc.tile_pool(name="p2sq", bufs=2))
        tmpp = ctx.enter_context(tc.tile_pool(name="p2tmp", bufs=1))

        for tt in range(NTT):
            y1T = inpool.tile([128, FC, TT], F32R, name="y1Ti", tag="y1Ti")
            for fc in range(FC):
                nc.sync.dma_start(
                    y1T[:, fc, :], y1_d[fc, :, tt * TT:(tt + 1) * TT])
            qgT = apool.tile([128, FC, TT], BF16, name="qgT", tag="qgT")
            _proj_fm(nc, ppsum, gqw, y1T, qgT, bias_ap=gqb)

            gAV = apool.tile([128, FC, TT], F32R, name="gAV", tag="gAV")
            gAVu = apool.tile([128, FC, TT], BF16, name="gAVu", tag="gAVu")
            zrows = rowp.tile([1, H, TT], F32, name="zrows", tag="zrows")
            for hh in range(H):
                pb, fc = _hrows(hh)
                emg = empool.tile([128, 4, TT], BF16, name="emg", tag="emg")
                for bc in range(4):
                    sps = ppsum.tile([128, TT], F32, name="gsp", tag="proj")
                    nc.tensor.matmul(
                        sps[:],
                        kTg[pb:pb + 64, fc, bc * 128:(bc + 1) * 128],
                        qgT[pb:pb + 64, fc, :],
                        start=True, stop=True)
                    nc.scalar.activation(emg[:, bc, :], sps[:], AF.Exp,
                                         scale=SCALE)
                avz = avpsum.tile([65, TT], F32, name="avz", tag="avz")
                for bc in range(4):
                    nc.tensor.matmul(avz[:], vg1[:, bc, hh, :], emg[:, bc, :],
                                     start=(bc == 0), stop=(bc == 3))
                nc.scalar.activation(zrows[0:1, hh, :], avz[64:65, :],
                                     AF.Identity)
                nc.any.tensor_copy(gAVu[pb:pb + 64, fc, :], avz[0:64, :])
            _tap(nc, "gAVu", gAVu[:].rearrange("p a b -> p (a b)"), tt)
            if TAP == "zrows_pre" and tt == 0:
                nc.sync.dma_start(
                    DBG_D.ap()[0:1, :],
                    zrows[0:1, 0:6, :].rearrange("p a b -> p (a b)"))
            rzb_b = rowp.tile([1, H, TT], BF16, name="rzb_b", tag="rzb_b")
            nc.scalar.activation(zrows[:], zrows[:], AF.Ln)
            nc.scalar.activation(rzb_b[:], zrows[:], AF.Exp, scale=-1.0)
            if TAP == "zrows_post" and tt == 0:
                nc.sync.dma_start(
                    DBG_D.ap()[0:1, 0:1536],
                    zrows[0:1, 0:3, :].rearrange("p a b -> p (a b)"))
            for fc in range(FC):
                # broadcast rZ rows for head pair (2fc, 2fc+1) via K=1
                # bf16 outer products into psum (partition moves need PE)
                rzps = avpsum.tile([128, TT], F32, name="rzps", tag="rzps",
                                   bufs=1)
                nc.vector.memset(rzps[:], 0.0)
                nc.tensor.matmul(rzps[0:64, :], ones64b[:],
                                 rzb_b[0:1, 2 * fc, :],
                                 start=False, stop=True,
                                 skip_group_check=True)
                nc.tensor.matmul(rzps[64:128, :], ones64b[:],
                                 rzb_b[0:1, 2 * fc + 1, :],
                                 start=False, stop=True,
                                 skip_group_check=True)
                nc.vector.tensor_tensor(
                    gAV[:, fc, :], gAVu[:, fc, :], rzps[:], op=OP.mult)

            _tap(nc, "qgT", qgT[:].rearrange("p a b -> p (a b)"), tt)
            _tap(nc, "gAV", gAV[:].rearrange("p a b -> p (a b)"), tt)
            r2T = apool.tile([128, FC, TT], F32R, name="r2T", tag="r2T")
            for fo in range(FC):
                ps = ppsum.tile([128, TT], F32, name="gop", tag="proj")
                for kc in range(FC):
                    nc.tensor.matmul(ps[:],
                                     gow[kc][:, fo * 128:(fo + 1) * 128],
                                     gAV[:, kc, :],
                                     start=(kc == 0), stop=(kc == FC - 1))
                t1 = tmpp.tile([128, TT], F32, name="got1", tag="got1")
                nc.scalar.activation(t1[:], ps[:], AF.Identity,
                                     bias=gobp[:, fo:fo + 1])
                nc.vector.tensor_tensor(r2T[:, fo, :], t1[:], y1T[:, fo, :],
                                        op=OP.add)

            _tap(nc, "r2T", r2T[:].rearrange("p a b -> p (a b)"), tt)
            y2T = apool.tile([128, FC, TT], F32R, name="y2T", tag="gAV")
            _ln_fm(nc, tc, (ppsum, rowp, sqp, bcp, tmpp), r2T, y2T,
                   n2g, n2b, ones_r, eps_row)
            _tap(nc, "y2T", y2T[:].rearrange("p a b -> p (a b)"), tt)
            for fc in range(FC):
                nc.sync.dma_start(
                    y2_d[fc, :, tt * TT:(tt + 1) * TT], y2T[:, fc, :])


def _phase3(nc, tc, octx, wd, y2_d, out_d, ident, ident_r, ones_r, eps_row,
            g3bc, b3bc):
    HC = 4 * D // 128  # 24 intermediate chunks
    with ExitStack() as ctx:
        wpool = ctx.enter_context(tc.tile_pool(name="p3w", bufs=1))
        inpool = ctx.enter_context(tc.tile_pool(name="p3in", bufs=1))
        hpool = ctx.enter_context(tc.tile_pool(name="p3h", bufs=2))
        apool = ctx.enter_context(tc.tile_pool(name="p3a", bufs=1))
        rowp = ctx.enter_context(tc.tile_pool(name="p3row", bufs=1))
        sqp = ctx.enter_context(tc.tile_pool(name="p3sq", bufs=2))
        tmpp = ctx.enter_context(tc.tile_pool(name="p3tmp", bufs=1))
        outp = ctx.enter_context(tc.tile_pool(name="p3out", bufs=1))
        fpsum = ctx.enter_context(
            tc.tile_pool(name="p3fp", bufs=1, space="PSUM"))
        spsum = ctx.enter_context(
            tc.tile_pool(name="p3sp", bufs=2, space="PSUM"))

        f1w, f2w = [], []
        with ExitStack() as wctx:
            stg = wctx.enter_context(tc.tile_pool(name="p3stg", bufs=2))
            for kc in range(FC):
                raw = stg.tile([128, 4 * D], F32, name="f1raw", tag="f1stg")
                nc.sync.dma_start(
                    raw[:], wd["f1_w"].ap()[kc * 128:(kc + 1) * 128, :]
                    .bitcast(F32))
                t = wpool.tile([128, 4 * D], BF16, name=f"f1w{kc}",
                               tag=f"f1w{kc}")
                nc.vector.tensor_copy(t[:], raw[:])
                f1w.append(t)
            for kc in range(HC):
                raw2 = stg.tile([128, D], F32, name="f2raw", tag="f2stg")
                nc.sync.dma_start(
                    raw2[:], wd["f2_w"].ap()[kc * 128:(kc + 1) * 128, :]
                    .bitcast(F32))
                t = wpool.tile([128, D], BF16, name=f"f2w{kc}",
                               tag=f"f2w{kc}")
                nc.vector.tensor_copy(t[:], raw2[:])
                f2w.append(t)
        f1b = _load_bias(nc, wpool, wd["f1_b"], HC, "f1b")
        f2b = _load_bias(nc, wpool, wd["f2_b"], FC, "f2b")

        for tt in range(NTT):
            y2T = inpool.tile([128, FC, TT], F32R, name="y2Ti", tag="y2Ti")
            for fc in range(FC):
                nc.sync.dma_start(
                    y2T[:, fc, :], y2_d[fc, :, tt * TT:(tt + 1) * TT])
            y2b = inpool.tile([128, FC, TT], BF16, name="y2b", tag="y2b")
            nc.vector.tensor_copy(y2b[:], y2T[:])

            fout = fpsum.tile([128, FC, TT], F32, name="fout", tag="fout")
            for hc in range(HC):
                ps = spsum.tile([128, TT], F32, name="f1p", tag="sblk")
                for kc in range(FC):
                    nc.tensor.matmul(ps[:],
                                     f1w[kc][:, hc * 128:(hc + 1) * 128],
                                     y2b[:, kc, :],
                                     start=(kc == 0), stop=(kc == FC - 1))
                hT = hpool.tile([128, TT], BF16, name="hT", tag="hT")
                nc.scalar.activation(hT[:], ps[:], AF.Gelu,
                                     bias=f1b[:, hc:hc + 1])
                for fo in range(FC):
                    nc.tensor.matmul(fout[:, fo, :],
                                     f2w[hc][:, fo * 128:(fo + 1) * 128],
                                     hT[:],
                                     start=(hc == 0), stop=(hc == HC - 1),
                                     skip_group_check=True)
            r3T = apool.tile([128, FC, TT], F32R, name="r3T", tag="r3T")
            Sps3 = spsum.tile([1, TT], F32, name="Sps3", tag="sblk")
            Qps3 = spsum.tile([1, TT], F32, name="Qps3", tag="sblk")
            for fo in range(FC):
                t1 = tmpp.tile([128, TT], F32, name="f2t1", tag="f2t1")
                nc.scalar.activation(t1[:], fout[:, fo, :], AF.Identity,
                                     bias=f2b[:, fo:fo + 1])
                nc.vector.tensor_tensor(r3T[:, fo, :], t1[:], y2T[:, fo, :],
                                        op=OP.add)
            for fo in range(FC):
                sq3 = sqp.tile([128, TT], F32R, name="sq3", tag="sq3",
                               bufs=1)
                nc.vector.tensor_tensor(sq3[:], r3T[:, fo, :], r3T[:, fo, :],
                                        op=OP.mult)
                nc.tensor.matmul(Sps3[:], ones_r[:], r3T[:, fo, :],
                                 start=(fo == 0), stop=(fo == FC - 1))
                nc.tensor.matmul(Qps3[:], ones_r[:], sq3[:],
                                 start=(fo == 0), stop=(fo == FC - 1))
            mu3r = rowp.tile([1, TT], F32, name="mu3r", tag="mu3r")
            nc.scalar.mul(mu3r[:], Sps3[:], 1.0 / D)
            msq3r = rowp.tile([1, TT], F32, name="msq3r", tag="msq3r")
            nc.scalar.mul(msq3r[:], Qps3[:], 1.0 / D)
            mu23r = rowp.tile([1, TT], F32, name="mu23r", tag="mu23r")
            nc.vector.tensor_tensor(mu23r[:], mu3r[:], mu3r[:], op=OP.mult)
            nc.vector.tensor_tensor(msq3r[:], msq3r[:], mu23r[:],
                                    op=OP.subtract)
            var3r = msq3r
            istd3r = rowp.tile([1, TT], F32, name="istd3r", tag="istd3r")
            nc.scalar.activation(var3r[:], var3r[:], AF.Ln,
                                 bias=eps_row[:])
            nc.scalar.activation(istd3r[:], var3r[:], AF.Exp, scale=-0.5)
            nc.vector.tensor_tensor(mu3r[:], mu3r[:], istd3r[:], op=OP.mult)
            nmu3r = mu3r
            # transpose the two [1, TT] rows into [128, 4] columns
            one1 = ident[0:1, 0:1]
            icol = rowp.tile([128, 2, 4], F32, name="icol", tag="icol")
            for tcb in range(4):
                rps = spsum.tile([128, 128], F32, name="rps", tag="sblk")
                nc.tensor.transpose(
                    rps[:, 0:1], istd3r[0:1, tcb * 128:(tcb + 1) * 128], one1)
                nc.tensor.transpose(
                    rps[:, 1:2], nmu3r[0:1, tcb * 128:(tcb + 1) * 128], one1)
                nc.any.tensor_copy(icol[:, 0, tcb:tcb + 1], rps[:, 0:1])
                nc.any.tensor_copy(icol[:, 1, tcb:tcb + 1], rps[:, 1:2])

            _tap(nc, "r3T", r3T[:].rearrange("p a b -> p (a b)"), tt)
            # transpose to token-major + LayerNorm3 + store
            for tcb in range(4):
                r3tm = apool.tile([128, D], F32, name="r3tm", tag="r3tm")
                for fc in range(FC):
                    tps = spsum.tile([128, 128], F32R, name="otr", tag="sblk")
                    nc.tensor.transpose(
                        tps[:], r3T[:, fc, tcb * 128:(tcb + 1) * 128],
                        ident_r[:])
                    nc.any.tensor_copy(r3tm[:, fc * 128:(fc + 1) * 128],
                                       tps[:].bitcast(F32))
                t1 = outp.tile([128, D], F32, name="o3t1", tag="o3t1",
                               bufs=2)
                nc.vector.tensor_scalar(
                    out=t1[:], in0=r3tm[:], scalar1=icol[:, 0, tcb:tcb + 1],
                    scalar2=icol[:, 1, tcb:tcb + 1],
                    op0=OP.mult, op1=OP.subtract)
                nc.vector.tensor_tensor(t1[:], t1[:], g3bc[:], op=OP.mult)
                nc.vector.tensor_tensor(t1[:], t1[:], b3bc[:], op=OP.add)
                nc.sync.dma_start(
                    out_d.ap()[tt * TT + tcb * 128:tt * TT + (tcb + 1) * 128,
                               :],
                    t1[:])


_NC = None


def _get_nc():
    global _NC
    if _NC is None:
        _NC = build()
    return _NC


BASS_INPUT_NAMES = [
    "bucket_matrix", "lq_w", "lq_b", "lk_w", "lv_w", "lo_w", "lo_bp",
    "gq_w", "gq_b", "gkv_w", "go_w", "go_bp", "f1_w", "f1_b", "f2_w", "f2_b",
    "n1_g", "n1_b", "n2_g", "n2_b", "n3_g", "n3_b",
]


def derived_inputs(inputs):
    f = lambda nm: np.asarray(inputs[nm], dtype=np.float32)
    d = {nm: f(nm) for nm in BASS_INPUT_NAMES if nm not in ("lo_bp", "go_bp")}
    d["lo_bp"] = f("lo_b") + f("lo_w").T @ f("lv_b")
    d["go_bp"] = f("go_b") + f("go_w").T @ f("gkv_b")[D:]
    return d


def kernel(**inputs):
    nc = _get_nc()
    x = np.asarray(inputs["x"], dtype=np.float32).reshape(B * S, D)
    weights = derived_inputs(inputs)
    in_maps = []
    for c in range(NCORES):
        m = {"x": np.ascontiguousarray(x[c * T:(c + 1) * T])}
        m.update(weights)
        in_maps.append(m)
    # First execution after NEFF load has shown rare, flaky corruption
    # (activation-table load race); run twice and return the second result.
    run_bass_kernel_spmd(nc, in_maps, list(range(NCORES)))
    res = run_bass_kernel_spmd(nc, in_maps, list(range(NCORES)))
    out = np.concatenate([res.results[c]["out"] for c in range(NCORES)],
                         axis=0)
    return out.reshape(B, S, D).astype(np.float32)
